# revision 29
# baseline (speedup 1.0000x reference)
"""Distributed QK-norm multi-head attention on 8 Trainium2 NeuronCores.

Strategy: tensor-parallel on heads (2 heads/core) through QKV projection and
attention; chunked AllGather of the head-major context (partition-axis
concat); column-parallel output projection. All operands are pre-transposed
and cast to bf16 on host so every matmul is in PE-native layout; f32
accumulation; softmax denominators via a ones-augmented V matmul.

Engine assignment keeps the Activation engine exp-pure during attention:
LN stats finalize with a single Rsqrt (no table thrash), PSUM evacuations run
on DVE, LN-apply on GpSimd, DMAs are merged (fixed ~625ns issue cost each)
and spread across the SP/DVE queues, and softmax normalization uses
reciprocal_approx_fast plus a PE rank-1 broadcast instead of the slow DVE
reciprocal + GpSimd partition_broadcast chain.

kernel(**inputs) takes the full unsharded inputs and returns the full
[2, 2048, 1024] float32 output.
"""

from contextlib import ExitStack

import numpy as np

import concourse.bass as bass
import concourse.bacc as bacc
import concourse.tile as tile
import concourse.mybir as mybir

F32 = mybir.dt.float32
I32 = mybir.dt.int32
BF16 = mybir.dt.bfloat16
FP8 = mybir.dt.float8e4
AF = mybir.ActivationFunctionType
OP = mybir.AluOpType

N_CORES = 8
B, NSEQ, D = 2, 2048, 1024
H, HD = 16, 64
HC = H // N_CORES          # heads per core = 2
T = B * NSEQ               # 4096 tokens
P = 128
NTB = T // P               # 32 token blocks
NTB_B = NSEQ // P          # 16 per batch half
ND = D // P                # 8 contraction tiles
KB = NSEQ // P             # 16 key blocks
QG = 512                   # q-group (moving free dim)
NQG = NSEQ // QG           # 4 q groups
EPS = 1e-5
W = 3 * P + 4              # 388: qkv outputs + 4 mean columns


def build(n_cores: int = N_CORES, trivial_gb: bool = True):
    nc = bacc.Bacc("TRN2", target_bir_lowering=False, debug=False,
                   num_devices=n_cores)

    xT = nc.dram_tensor("xT", [D, T], BF16, kind="ExternalInput")
    wqkvT = nc.dram_tensor("wqkvT", [D, W], BF16, kind="ExternalInput")
    bqkv = nc.dram_tensor("bqkv", [1, W], BF16, kind="ExternalInput")
    wpT = nc.dram_tensor("wpT", [D, P], BF16, kind="ExternalInput")
    bp = nc.dram_tensor("bp", [P, 1], F32, kind="ExternalInput")
    qg2 = nc.dram_tensor("qg2", [P, 1], F32, kind="ExternalInput")
    qb2 = nc.dram_tensor("qb2", [P, 1], F32, kind="ExternalInput")
    kg2 = nc.dram_tensor("kg2", [P, 1], F32, kind="ExternalInput")
    kb2 = nc.dram_tensor("kb2", [P, 1], F32, kind="ExternalInput")
    ident = nc.dram_tensor("ident", [P, P], BF16, kind="ExternalInput")
    outT = nc.dram_tensor("outT", [P, T], F32, kind="ExternalOutput")

    with tile.TileContext(nc) as tc, ExitStack() as ctx:
        pools = {}
        for name, bufs, space in [
            ("xt", 1, "SBUF"), ("wq", 1, "SBUF"), ("wp", 1, "SBUF"),
            ("const", 1, "SBUF"), ("qkt", 1, "SBUF"), ("vp", 1, "SBUF"),
            ("raw", 1, "SBUF"), ("stat", 1, "SBUF"), ("sq", 2, "SBUF"),
            ("tok", 4, "SBUF"), ("at", 3, "SBUF"), ("rb", 4, "SBUF"),
            ("cstage", 1, "SBUF"), ("pr", 4, "SBUF"),
            ("osb", 2, "SBUF"), ("dram", 1, "DRAM"),
            ("ps_misc", 2, "PSUM"), ("ps_s", 2, "PSUM"), ("ps_ctx", 2, "PSUM"),
        ]:
            pools[name] = ctx.enter_context(
                tc.tile_pool(name=name, bufs=bufs, space=space))

        # ---- persistent SBUF tensors ----
        # xt as one tile so chunk loads merge into few wide DMAs
        xt_all = pools["xt"].tile([P, ND, T], BF16, name="xt_all")

        def load_xt_chunk(ch, nway=2):
            # chunk ch: xT[:, ch*QG:(ch+1)*QG] -> xt_all[:, :, ch], nway DMAs
            eng_l = [nc.sync, nc.scalar]
            step = ND // nway
            for i in range(nway):
                src = xT[i * step * P:(i + 1) * step * P,
                         ch * QG:(ch + 1) * QG]
                eng_l[i % 2].dma_start(
                    xt_all[:, i * step:(i + 1) * step,
                           ch * QG:(ch + 1) * QG],
                    src.rearrange("(dt p) q -> p dt q", p=P))

        wq_all = pools["wq"].tile([P, ND, W], BF16, name="wq_all")

        # startup: first xt chunk split 4-way for latency, weights alongside
        load_xt_chunk(0, nway=8)
        for dt in range(ND):
            eng = nc.gpsimd if dt % 2 == 0 else nc.scalar
            eng.dma_start(wq_all[:, dt, :],
                          wqkvT[dt * P:(dt + 1) * P, :])
        load_xt_chunk(1)

        wp_all = pools["wp"].tile([P, ND, P], BF16, name="wp_all")
        nc.gpsimd.dma_start(
            wp_all[:], wpT[:].rearrange("(dt p) q -> p dt q", p=P))

        cp = pools["const"]
        bqkv_sb = cp.tile([1, W], BF16, name="bqkv_sb")
        nc.sync.dma_start(bqkv_sb[:], bqkv[:])
        bp_sb = cp.tile([P, 1], F32, name="bp_sb")
        nc.sync.dma_start(bp_sb[:], bp[:])
        gb_sb = {}
        for nm, src in (("qg2", qg2), ("qb2", qb2), ("kg2", kg2), ("kb2", kb2)):
            t_ = cp.tile([P, 1], F32, name=f"{nm}_sb")
            nc.sync.dma_start(t_[:], src[:])
            gb_sb[nm] = t_
        ident_sb = cp.tile([P, P], BF16, name="ident_sb")
        nc.sync.dma_start(ident_sb[:], ident[:])
        ones_sb = cp.tile([1, P], BF16, name="ones_sb")
        nc.vector.memset(ones_sb[:], 1.0)
        onesc_sb = cp.tile([1, HD], BF16, name="onesc_sb")
        nc.vector.memset(onesc_sb[:], 1.0)
        eps_sb = cp.tile([P, 1], F32, name="eps_sb")
        nc.vector.memset(eps_sb[:], EPS)
        zero_sb = cp.tile([P, 1], F32, name="zero_sb")
        nc.vector.memset(zero_sb[:], 0.0)

        qt_sb = [pools["qkt"].tile([P, NSEQ], BF16, name=f"qt{b}") for b in range(B)]
        kt_sb = [pools["qkt"].tile([P, NSEQ], BF16, name=f"kt{b}") for b in range(B)]
        HDP = HD + 1   # head pitch: 64 V dims + ones column
        vp_sb = [pools["vp"].tile([P, KB, HC * HDP], BF16, name=f"vp{b}")
                 for b in range(B)]
        for b in range(B):
            for h in range(HC):
                nc.vector.memset(
                    vp_sb[b][:, :, h * HDP + HD:h * HDP + HD + 1], 1.0)
        qkraw = [pools["raw"].tile([P, NTB_B, 2 * P], BF16, name=f"qkraw{b}")
                 for b in range(B)]
        svar = [pools["stat"].tile([P, 4 * NTB_B], F32, name=f"svar{b}")
                for b in range(B)]
        smu = [pools["stat"].tile([P, 4 * NTB_B], F32, name=f"smu{b}")
               for b in range(B)]
        nmurs = [pools["stat"].tile([P, 4 * NTB_B], F32, name=f"nmurs{b}")
                 for b in range(B)]
        rstd_all = [pools["stat"].tile([P, 4 * NTB_B], F32, name=f"rstd{b}")
                    for b in range(B)]
        cstage = [pools["cstage"].tile([P, NSEQ], BF16, name=f"cstage{b}")
                  for b in range(B)]

        warm_in = pools["dram"].tile([P, 4], BF16, name="warm_in")
        warm_out = pools["dram"].tile([P * n_cores, 4], BF16, name="warm_out",
                                      addr_space="Shared")
        warm_sb = cp.tile([P, 4], BF16, name="warm_sb")
        nc.vector.memset(warm_sb[:], 0.0)
        nc.sync.dma_start(warm_in[:], warm_sb[:])
        nc.gpsimd.collective_compute(
            "AllGather", OP.bypass, replica_groups=[list(range(n_cores))],
            ins=[warm_in[:].opt()], outs=[warm_out[:].opt()])

        bounce = [[pools["dram"].tile([P, QG], BF16, name=f"bounce{b}_{qg}")
                   for qg in range(NQG)] for b in range(B)]
        gathered = [[pools["dram"].tile([P * n_cores, QG], BF16,
                                        name=f"gath{b}_{qg}",
                                        addr_space="Shared")
                     for qg in range(NQG)] for b in range(B)]

        QTR = 4                      # token blocks per stats group
        NQTR = NTB // QTR            # 8 groups across both halves

        def b1_quarter(q):
            if q + 2 < NQTR:
                load_xt_chunk(q + 2)
            for tb in range(q * QTR, (q + 1) * QTR):
                b, tb_b = divmod(tb, NTB_B)
                psp = pools["ps_misc"].tile([P, W], F32, name="psp", tag="misc")
                for dt in range(ND):
                    nc.tensor.matmul(psp[:],
                                     xt_all[:, dt, tb * P:(tb + 1) * P],
                                     wq_all[:, dt, :], start=(dt == 0),
                                     stop=False)
                nc.tensor.matmul(psp[:], ones_sb[:], bqkv_sb[:],
                                 start=False, stop=True)
                nc.vector.tensor_copy(qkraw[b][:, tb_b, :], psp[:, 0:2 * P])
                nc.vector.tensor_copy(
                    vp_sb[b][:, tb_b, :].rearrange("p (h w) -> p h w",
                                                   h=HC)[:, :, 0:HD],
                    psp[:, 2 * P:3 * P].rearrange("p (h w) -> p h w", h=HC))
                nc.vector.tensor_copy(smu[b][:, 4 * tb_b:4 * tb_b + 4],
                                      psp[:, 3 * P:W])
                sq = pools["sq"].tile([P, 2 * P], F32, name="sq", tag="sq")
                nc.vector.tensor_tensor(out=sq[:], in0=qkraw[b][:, tb_b, :],
                                        in1=qkraw[b][:, tb_b, :], op=OP.mult)
                nc.vector.tensor_reduce(
                    svar[b][:, 4 * tb_b:4 * tb_b + 4],
                    sq[:].rearrange("p (g w) -> p g w", g=4),
                    axis=mybir.AxisListType.X, op=OP.add)
            # stats chain for this quarter -> rstd slice (single Rsqrt)
            b, q_b = divmod(q, NTB_B // QTR)
            lo, hi = 4 * q_b * QTR, 4 * (q_b + 1) * QTR
            varb = pools["sq"].tile([P, 4 * QTR], F32, name="varb", tag="varb")
            musq = pools["sq"].tile([P, 4 * QTR], F32, name="musq", tag="musq")
            nc.vector.tensor_tensor(out=musq[:], in0=smu[b][:, lo:hi],
                                    in1=smu[b][:, lo:hi], op=OP.mult)
            nc.vector.tensor_scalar(varb[:], svar[b][:, lo:hi], 1.0 / HD, None,
                                    op0=OP.mult)
            nc.vector.tensor_tensor(out=varb[:], in0=varb[:], in1=musq[:],
                                    op=OP.subtract)
            # rstd = rsqrt(var+eps) entirely on DVE (Quake seed + 2 Newton
            # steps) so the Act engine's exp table is never displaced.
            nc.vector.tensor_scalar(varb[:], varb[:], EPS, None, op0=OP.add)
            yt = pools["sq"].tile([P, 4 * QTR], F32, name="yt", tag="yt")
            nc.vector.tensor_scalar(
                yt[:].bitcast(I32), varb[:].bitcast(I32), 1, None,
                op0=OP.logical_shift_right)
            nc.vector.tensor_scalar(
                yt[:].bitcast(I32), yt[:].bitcast(I32), -1, 0x5f3759df,
                op0=OP.mult, op1=OP.add)
            for _ in range(2):
                y2 = pools["sq"].tile([P, 4 * QTR], F32, name="y2", tag="y2")
                nc.vector.tensor_tensor(out=y2[:], in0=yt[:], in1=yt[:],
                                        op=OP.mult)
                nc.vector.tensor_tensor(out=y2[:], in0=y2[:], in1=varb[:],
                                        op=OP.mult)
                nc.vector.tensor_scalar(y2[:], y2[:], -0.5, 1.5, op0=OP.mult,
                                        op1=OP.add)
                nc.vector.tensor_tensor(out=yt[:], in0=yt[:], in1=y2[:],
                                        op=OP.mult)
            nc.vector.tensor_copy(rstd_all[b][:, lo:hi], yt[:])
            # -mu * rstd, so LN-apply is a single fused multiply-add
            nc.vector.tensor_tensor(out=nmurs[b][:, lo:hi],
                                    in0=smu[b][:, lo:hi],
                                    in1=rstd_all[b][:, lo:hi], op=OP.mult)
            nc.vector.tensor_scalar(nmurs[b][:, lo:hi], nmurs[b][:, lo:hi],
                                    -1.0, None, op0=OP.mult)

        def b2_quarter(q):
            b, q_b = divmod(q, NTB_B // QTR)
            for tb_b in range(q_b * QTR, (q_b + 1) * QTR):
                for qk, (g2, b2, dst) in enumerate((
                        (gb_sb["qg2"], gb_sb["qb2"], qt_sb[b]),
                        (gb_sb["kg2"], gb_sb["kb2"], kt_sb[b]))):
                    tokt = pools["tok"].tile([P, P], BF16, name="tokt",
                                             tag="tok")
                    for h in range(HC):
                        i = 4 * tb_b + 2 * qk + h
                        # x*rstd + (-mu*rstd) on gpsimd, freeing DVE/Act
                        nc.gpsimd.tensor_scalar(
                            tokt[:, h * HD:(h + 1) * HD],
                            qkraw[b][:, tb_b,
                                     qk * P + h * HD:qk * P + (h + 1) * HD],
                            rstd_all[b][:, i:i + 1], nmurs[b][:, i:i + 1],
                            op0=OP.mult, op1=OP.add)
                    pst = pools["ps_misc"].tile([P, P], BF16, name="pst",
                                                tag="misc")
                    nc.tensor.transpose(pst[:], tokt[:], ident_sb[:])
                    if trivial_gb:
                        nc.vector.tensor_copy(
                            dst[:, tb_b * P:(tb_b + 1) * P], pst[:])
                    else:
                        nc.vector.tensor_scalar(
                            dst[:, tb_b * P:(tb_b + 1) * P], pst[:],
                            g2[:], b2[:], op0=OP.mult, op1=OP.add)

        def scores(b, qg, kb, pss):
            for h in range(HC):
                nc.tensor.matmul(
                    pss[:, h * QG:(h + 1) * QG],
                    kt_sb[b][h * HD:(h + 1) * HD, kb * P:(kb + 1) * P],
                    qt_sb[b][h * HD:(h + 1) * HD, qg * QG:(qg + 1) * QG],
                    start=True, stop=True)

        prt_sb = {}

        def issue_prt(b, qg, nway=1):
            # prefetch gathered context into SBUF as two half tiles so the
            # projection can start once the first half lands
            halves = []
            for j in range(2):
                t_ = pools["pr"].tile([P, ND // 2, QG], BF16, name="prt",
                                      tag="pr")
                step = max(ND // 2 // nway, 1)
                for i in range(ND // 2 // step):
                    lo = j * (ND // 2) + i * step
                    src = gathered[b][qg][lo * P:(lo + step) * P, :]
                    nc.sync.dma_start(
                        t_[:, i * step:(i + 1) * step, :],
                        src.rearrange("(dt p) q -> p dt q", p=P))
                halves.append(t_)
            prt_sb[(b, qg)] = halves

        def phase_d(b, qg):
            halves = prt_sb.pop((b, qg))
            pso = pools["ps_misc"].tile([P, QG], F32, name="pso", tag="misc")
            for dt in range(n_cores):
                nc.tensor.matmul(pso[:], wp_all[:, dt, :],
                                 halves[dt // (ND // 2)][:, dt % (ND // 2), :],
                                 start=(dt == 0), stop=(dt == n_cores - 1))
            osb = pools["osb"].tile([P, QG], F32, name="osb", tag="osb")
            nc.vector.tensor_scalar(osb[:], pso[:], bp_sb[:], None, op0=OP.add)
            nc.sync.dma_start(
                outT[:, b * NSEQ + qg * QG:b * NSEQ + (qg + 1) * QG], osb[:])

        def phase_c_qg(b, qg, drains):
            ctx_ps = [pools["ps_ctx"].tile([HD + 1, QG], F32, name="ctx",
                                           tag="ctx") for _ in range(HC)]
            pss_tiles = {}
            pss_tiles[0] = pools["ps_s"].tile([P, 2 * QG], F32, name="pss",
                                              tag="pss")
            scores(b, qg, 0, pss_tiles[0])
            for kb in range(KB):
                if kb + 1 < KB:
                    pss_tiles[kb + 1] = pools["ps_s"].tile(
                        [P, 2 * QG], F32, name="pss", tag="pss")
                    scores(b, qg, kb + 1, pss_tiles[kb + 1])
                at = pools["at"].tile([P, 2 * QG], BF16, name="at", tag="at")
                nc.scalar.activation(at[:], pss_tiles.pop(kb)[:], AF.Exp,
                                     bias=zero_sb[:], scale=0.125)
                for h in range(HC):
                    nc.tensor.matmul(
                        ctx_ps[h][:],
                        vp_sb[b][:, kb, h * HDP:h * HDP + HD + 1],
                        at[:, h * QG:(h + 1) * QG],
                        start=(kb == 0), stop=(kb == KB - 1))
                # spread drain work across the kb loop
                if kb == 4 and drains and drains[0] is not None:
                    issue_prt(*drains[0][1:]) if drains[0][0] == "prt" \
                        else phase_d(*drains[0][1:])
                if kb == 10 and len(drains) > 1 and drains[1] is not None:
                    issue_prt(*drains[1][1:]) if drains[1][0] == "prt" \
                        else phase_d(*drains[1][1:])
            # softmax normalization: fast reciprocal + PE rank-1 broadcast.
            # The custom-DVE reciprocal misreads inputs whose base partition
            # is nonzero, so the denominator row is staged to partition 0.
            ctxs_l, recs = [], []
            for h in range(HC):
                ctxs = pools["rb"].tile([HD + 1, QG], F32, name="ctxs",
                                        tag="ctxs")
                nc.vector.tensor_copy(ctxs[:], ctx_ps[h][0:HD + 1, :])
                ctxs_l.append(ctxs)
                den = pools["rb"].tile([1, QG], F32, name="den", tag="den")
                nc.vector.tensor_copy(den[:], ctxs[HD:HD + 1, :])
                rec = pools["rb"].tile([1, QG], F32, name="rec", tag="rec")
                nc.vector.reciprocal_approx_fast(out=rec[:], in_=den[:])
                recs.append(rec)
            for h in range(HC):
                rb = pools["rb"].tile([HD, QG], F32, name="rb", tag="rb")
                nc.gpsimd.partition_broadcast(rb[:], recs[h][:])
                nc.vector.tensor_tensor(
                    out=cstage[b][h * HD:(h + 1) * HD,
                                  qg * QG:(qg + 1) * QG],
                    in0=ctxs_l[h][0:HD, :], in1=rb[:], op=OP.mult)
            for i in range(2):
                nc.sync.dma_start(
                    bounce[b][qg][:, i * (QG // 2):(i + 1) * (QG // 2)],
                    cstage[b][:, qg * QG + i * (QG // 2):
                              qg * QG + (i + 1) * (QG // 2)])
            nc.gpsimd.collective_compute(
                "AllGather", OP.bypass,
                replica_groups=[list(range(n_cores))],
                ins=[bounce[b][qg][:].opt()],
                outs=[gathered[b][qg][:].opt()])

        # batch-0 head phases first, then interleave batch-1 head phases
        # with batch-0 attention so the in-order PE queue alternates QKV and
        # score/ctx work instead of serializing all QKV first.
        for q in range(4):
            b1_quarter(q)
            if q >= 1:
                b2_quarter(q - 1)
        b2_quarter(3)

        # drain schedule: prt issue ~1 collective after the allgather,
        # projection ~2 after; all drain trafficking off the Act queue.
        sched = {
            (0, 1): [("prt", 0, 0)],
            (0, 2): [("mm", 0, 0), ("prt", 0, 1)],
            (0, 3): [("mm", 0, 1), ("prt", 0, 2)],
            (1, 0): [("mm", 0, 2), ("prt", 0, 3)],
            (1, 1): [("mm", 0, 3)],
            (1, 2): [("prt", 1, 0)],
            (1, 3): [("mm", 1, 0), ("prt", 1, 1)],
        }
        for qg in range(NQG):
            phase_c_qg(0, qg, sched.get((0, qg), []))
            b1_quarter(qg + 4)
            if qg >= 1:
                b2_quarter(qg + 3)
        b2_quarter(NQTR - 1)
        for qg in range(NQG):
            phase_c_qg(1, qg, sched.get((1, qg), []))
        phase_d(1, 1)
        issue_prt(1, 2, nway=4)
        phase_d(1, 2)
        issue_prt(1, 3, nway=4)
        phase_d(1, 3)

    nc.compile()
    return nc


def prep_inputs(inputs):
    """Host-side prep: slice/transpose/cast per core. Returns (in_maps, trivial_gb)."""
    import ml_dtypes
    bf16 = ml_dtypes.bfloat16

    q = np.asarray(inputs["query"], np.float32)
    Wq, Wk, Wv, Wp = (np.asarray(inputs[k], np.float32)
                      for k in ("Wq", "Wk", "Wv", "Wp"))
    bq, bk, bv, bpv = (np.asarray(inputs[k], np.float32)
                       for k in ("bq", "bk", "bv", "bp"))
    qg, qb, kg, kb = (np.asarray(inputs[k], np.float32)
                      for k in ("q_gamma", "q_beta", "k_gamma", "k_beta"))

    trivial_gb = bool(
        np.all(qg == 1.0) and np.all(kg == 1.0)
        and np.all(qb == 0.0) and np.all(kb == 0.0))

    xT = np.ascontiguousarray(q.reshape(T, D).T).astype(bf16)
    identity = np.eye(P, dtype=bf16)
    in_maps = []
    for c in range(N_CORES):
        sl = slice(c * P, (c + 1) * P)
        wq_c, wk_c, wv_c = Wq[sl].T, Wk[sl].T, Wv[sl].T  # [1024, 128] each
        mean_cols = np.stack([
            wq_c[:, 0:HD].mean(axis=1), wq_c[:, HD:2 * HD].mean(axis=1),
            wk_c[:, 0:HD].mean(axis=1), wk_c[:, HD:2 * HD].mean(axis=1),
        ], axis=1)                                        # [1024, 4]
        wqkvT = np.concatenate([wq_c, wk_c, wv_c, mean_cols],
                               axis=1).astype(bf16)       # [1024, 388]
        bq_c, bk_c, bv_c = bq[sl], bk[sl], bv[sl]
        bias_means = np.array([
            bq_c[0:HD].mean(), bq_c[HD:].mean(),
            bk_c[0:HD].mean(), bk_c[HD:].mean()], np.float32)
        bqkv = np.concatenate([bq_c, bk_c, bv_c, bias_means])[None, :].astype(bf16)
        in_maps.append({
            "xT": xT,
            "wqkvT": np.ascontiguousarray(wqkvT),
            "bqkv": np.ascontiguousarray(bqkv),
            "wpT": np.ascontiguousarray(Wp[sl].T).astype(bf16),
            "bp": np.ascontiguousarray(bpv[sl].reshape(P, 1)),
            "qg2": np.tile(qg, HC).reshape(P, 1).astype(np.float32),
            "qb2": np.tile(qb, HC).reshape(P, 1).astype(np.float32),
            "kg2": np.tile(kg, HC).reshape(P, 1).astype(np.float32),
            "kb2": np.tile(kb, HC).reshape(P, 1).astype(np.float32),
            "ident": identity,
        })
    return in_maps, trivial_gb


def assemble_output(results):
    outT = np.concatenate([np.asarray(r["outT"], np.float32) for r in results],
                          axis=0)           # [1024, 4096]
    return np.ascontiguousarray(outT.T).reshape(B, NSEQ, D)


_CACHE = {}


def kernel(**inputs):
    from concourse.bass_utils import run_bass_kernel_spmd

    in_maps, trivial = prep_inputs(inputs)
    key = ("nc", trivial)
    if key not in _CACHE:
        _CACHE[key] = build(trivial_gb=trivial)
    nc = _CACHE[key]
    res = run_bass_kernel_spmd(nc, in_maps, core_ids=list(range(N_CORES)))
    return assemble_output(res.results)
